# revision 1
# baseline (speedup 1.0000x reference)
"""Trainium2 Bass kernel for the two-stream LSTM encoder.

Strategy (8 NeuronCores): the two LSTM streams are independent recurrences
(the cross-stream gating is output-only), so cores are paired: each core
runs ONE stream for its pair's 32 batch rows. This halves the per-core
recurrent weight streaming (the dominant PE cost: w_hh must flow through
the PE array every step).

  - The program is identical on all cores (SPMD): stream identity enters
    only through per-core input tensors. Features and w_emb are zero-padded
    to a common 2048 width so both streams share one shape; a 0/1 `ssel`
    input drives DVE/ACT selects for the BN stat slots and the batch half
    at gating time.
  - Phase A: each core embeds ITS stream for all 32 pair-batch rows
    (z = featS @ w_embS.T, PE-transposed into T-layout, spilled to DRAM
    fp16). BN stats are all-reduced over all 8 cores into per-stream slots.
  - Phase B: time loop in chunks of 16 steps, one stream, batch 32 on the
    free dim. Per chunk the h history is exchanged within the pair
    (AllGather via DRAM); gating + fusion + output for chunk c-1 are issued
    AFTER the recurrence of chunk c so the collective latency and the
    gating matmuls hide inside the next chunk's recurrence.
Gate order is host-permuted from torch's (i,f,g,o) to (i,f,o,g) so one
sigmoid ACT covers a contiguous [128,192] region and one tanh covers [128,64].
"""

import os
import numpy as np

os.environ.setdefault("MYCRO_LOCAL_CACHE", "1")

NCORES = 8
B, T, D, F0, F1 = 128, 256, 512, 2048, 1024
FR = 2048                 # padded feature width (shared by both streams)
BS = B // NCORES          # 16 batch rows per core for output
BSP = 2 * BS              # 32 batch rows per pair (recurrence batch)
ROWS = BS * T
TC = 16                   # time steps per chunk
NCHUNK = T // TC
EPS = 1e-5
G4 = 4 * D                # 2048 gate dim
NDC = D // 128            # 4 d-chunks
NGC = G4 // 128           # 16 gate chunks
NFC = FR // 128           # 16 feature chunks

PAIRS = [[2 * i, 2 * i + 1] for i in range(NCORES // 2)]

_BUILT = None


def _build(t_steps=T, use_collective=True):
    import concourse.bass as bass
    import concourse.bacc as bacc
    import concourse.mybir as mybir
    import concourse.tile as tile
    from concourse.masks import make_identity
    from contextlib import ExitStack

    f16 = mybir.dt.float16
    f32 = mybir.dt.float32
    AF = mybir.ActivationFunctionType
    ALU = mybir.AluOpType

    nchunk = t_steps // TC
    rows2 = BSP * t_steps     # 8192 pair rows
    nrc2 = rows2 // 128       # 64 row chunks

    nc = bacc.Bacc(None, num_devices=NCORES)

    # ---------------- DRAM parameters ----------------
    featS = nc.declare_dram_parameter("featS", [rows2, FR], f32, isOutput=False)
    maskp = nc.declare_dram_parameter("feat_mask", [BSP, t_steps], f32, isOutput=False)
    wembp = nc.declare_dram_parameter("w_embST", [FR, D], f16, isOutput=False)
    wihp = nc.declare_dram_parameter("w_ihST", [D, G4], f16, isOutput=False)
    whhp = nc.declare_dram_parameter("w_hhST", [D, G4], f16, isOutput=False)
    wg0T = nc.declare_dram_parameter("wg0T", [D, D], f16, isOutput=False)
    wg1T = nc.declare_dram_parameter("wg1T", [D, D], f16, isOutput=False)
    wf1T = nc.declare_dram_parameter("wf1T", [D, D], f16, isOutput=False)
    wf2T = nc.declare_dram_parameter("wf2T", [D, D], f16, isOutput=False)
    bcp = nc.declare_dram_parameter("bcS", [G4], f32, isOutput=False)
    bg0p = nc.declare_dram_parameter("bg0", [1, D], f16, isOutput=False)
    bg1p = nc.declare_dram_parameter("bg1", [1, D], f16, isOutput=False)
    bfp = nc.declare_dram_parameter("bf", [1, D], f16, isOutput=False)
    gammap = nc.declare_dram_parameter("gammaS", [D], f32, isOutput=False)
    betap = nc.declare_dram_parameter("betaS", [D], f32, isOutput=False)
    sselp = nc.declare_dram_parameter("ssel", [128], f32, isOutput=False)
    nsselp = nc.declare_dram_parameter("nssel", [128], f32, isOutput=False)
    out_d = nc.declare_dram_parameter("out", [BS, t_steps, D], f32, isOutput=True)

    # DRAM scratch
    zin = nc.dram_tensor("zin", [NDC, 128, rows2], f16)
    st_in = nc.dram_tensor("st_in", [128, 16], f32)
    st_out = nc.dram_tensor("st_out", [128, 16], f32)
    hin_d = [nc.dram_tensor(f"hin{i}", [128, TC, NDC, BSP], f16)
             for i in range(2)]
    hga_d = [nc.dram_tensor(f"hga{i}", [2, 128, TC, NDC, BSP], f16)
             for i in range(2)]

    inv_n = 1.0 / float(B * t_steps)
    if not use_collective:
        inv_n = 1.0 / float(BSP * t_steps)

    with tile.TileContext(nc) as tc, ExitStack() as stk:
        wp = stk.enter_context(tc.tile_pool(name="wp", bufs=1))

        # ---------------- resident small tiles ----------------
        ident = wp.tile([128, 128], f16)
        make_identity(nc, ident)
        ones16 = wp.tile([1, 512], f16)
        nc.vector.memset(ones16, 1.0)
        eps_t = wp.tile([128, 1], f32)
        nc.vector.memset(eps_t, 0.0)
        ssel = wp.tile([128, 1], f32)
        nc.sync.dma_start(out=ssel, in_=sselp.rearrange("(p o) -> p o", o=1))
        nssel = wp.tile([128, 1], f32)
        nc.sync.dma_start(out=nssel, in_=nsselp.rearrange("(p o) -> p o", o=1))

        def load_T(dram, nch, free):
            t = wp.tile([128, nch, free], f16, name=f"w_{dram.name}")
            nc.sync.dma_start(
                out=t, in_=dram.rearrange("(c p) g -> p c g", p=128))
            return t

        wih = load_T(wihp, NDC, G4)
        whh = load_T(whhp, NDC, G4)
        wg0 = load_T(wg0T, NDC, D)
        wg1 = load_T(wg1T, NDC, D)
        wf1 = load_T(wf1T, NDC, D)
        wf2 = load_T(wf2T, NDC, D)
        # cell bias in column layout [128, NGC] (per-gate-channel, consumed
        # via the ACT bias port during the u copy-out)
        bcc = wp.tile([128, NGC], f32)
        nc.sync.dma_start(out=bcc, in_=bcp.rearrange("(g p) -> p g", p=128))
        bg0 = wp.tile([1, D], f16)
        nc.sync.dma_start(out=bg0, in_=bg0p[:, :])
        bg1 = wp.tile([1, D], f16)
        nc.sync.dma_start(out=bg1, in_=bg1p[:, :])
        bf = wp.tile([1, D], f16)
        nc.sync.dma_start(out=bf, in_=bfp[:, :])

        gamS = wp.tile([128, NDC], f32)
        nc.sync.dma_start(out=gamS, in_=gammap.rearrange("(c p) -> p c", p=128))
        betS = wp.tile([128, NDC], f32)
        nc.sync.dma_start(out=betS, in_=betap.rearrange("(c p) -> p c", p=128))

        # pair mask (fp16, consumed by K=1 broadcast matmuls per chunk)
        mflat = wp.tile([1, BSP, t_steps], f16)

        # per-rowgroup stat slots for MY stream: col = dc*nrcg + rc
        nrcg = nrc2 // 2
        stat_s = wp.tile([128, NDC * nrcg], f32, name="stat_s")
        stat_q = wp.tile([128, NDC * nrcg], f32, name="stat_q")

        # ---------------- Phase A: my stream, 32 pair-batch rows ----------
        with tc.tile_pool(name="pa", bufs=2) as pa, \
             tc.tile_pool(name="paw", bufs=1) as paw, \
             tc.tile_pool(name="psA", bufs=1, space="PSUM") as psA:
            wemb = paw.tile([128, NFC, D], f16)
            nc.sync.dma_start(
                out=wemb, in_=wembp.rearrange("(c p) g -> p c g", p=128))
            mflat32 = paw.tile([1, BSP, t_steps], f32)
            nc.sync.dma_start(out=mflat32[0:1], in_=maskp[:, :])
            nc.vector.tensor_copy(mflat[0:1], mflat32[0:1])

            RB = 2   # row-chunks batched per matmul group (free dim 256)
            for rc in range(nrc2 // RB):
                ftile = pa.tile([128, RB, FR], f32, tag="ft")
                for j in range(RB):
                    nc.gpsimd.dma_start(
                        out=ftile[:, j],
                        in_=featS[(RB * rc + j) * 128:
                                  (RB * rc + j + 1) * 128, :])
                f16t = pa.tile([128, RB, FR], f16, tag="f16")
                nc.scalar.copy(f16t, ftile)
                fT = pa.tile([128, NFC, RB * 128], f16, tag="fT")
                for fc in range(NFC):
                    for j in range(RB):
                        tp = psA.tile([128, 128], f16, tag="tp", bufs=4)
                        nc.tensor.transpose(
                            tp, f16t[:, j, fc * 128:(fc + 1) * 128], ident)
                        nc.vector.tensor_copy(
                            fT[:, fc, j * 128:(j + 1) * 128], tp)
                za = psA.tile([128, NDC, RB * 128], f32, tag="za", bufs=2)
                for dc in range(NDC):
                    for fc in range(NFC):
                        nc.tensor.matmul(
                            za[:, dc],
                            lhsT=wemb[:, fc, dc * 128:(dc + 1) * 128],
                            rhs=fT[:, fc],
                            start=(fc == 0), stop=(fc == NFC - 1))
                zst = pa.tile([128, NDC, RB * 128], f16, tag="zst")
                sq = pa.tile([128, RB * 128], f32, tag="sq")
                for dc in range(NDC):
                    nc.scalar.activation(
                        zst[:, dc], za[:, dc],
                        AF.Identity,
                        accum_out=stat_s[:, dc * nrcg + rc:dc * nrcg + rc + 1])
                    nc.vector.tensor_tensor(
                        sq, za[:, dc], zst[:, dc], op=ALU.mult)
                    nc.vector.reduce_sum(
                        stat_q[:, dc * nrcg + rc:dc * nrcg + rc + 1],
                        sq, axis=mybir.AxisListType.X)
                nc.gpsimd.dma_start(
                    out=zin[:, :, rc * RB * 128:(rc + 1) * RB * 128].rearrange(
                        "c p r -> p c r"),
                    in_=zst)

        # ---------------- BN stats allreduce (per-stream slots) -----------
        ared = wp.tile([128, 8], f32)
        nc.vector.reduce_sum(
            ared[:, 0:4],
            stat_s.rearrange("p (c r) -> p c r", c=NDC),
            axis=mybir.AxisListType.X)
        nc.vector.reduce_sum(
            ared[:, 4:8],
            stat_q.rearrange("p (c r) -> p c r", c=NDC),
            axis=mybir.AxisListType.X)
        # slot 0:8 <- stream0 contribution, 8:16 <- stream1 (ssel-masked)
        sts = wp.tile([128, 16], f32)
        nc.scalar.activation(sts[:, 0:8], ared, AF.Identity,
                             scale=nssel[:, 0:1])
        nc.scalar.activation(sts[:, 8:16], ared, AF.Identity,
                             scale=ssel[:, 0:1])
        nc.gpsimd.dma_start(out=st_in[:, :], in_=sts)
        if use_collective:
            nc.gpsimd.collective_compute(
                "AllReduce", ALU.add,
                replica_groups=[list(range(NCORES))],
                ins=[st_in[:, :]], outs=[st_out[:, :]])
        else:
            nc.gpsimd.dma_start(out=st_out[:, :], in_=st_in[:, :])
        ag = wp.tile([128, 16], f32)
        nc.gpsimd.dma_start(out=ag, in_=st_out[:, :])

        # my stream's stats: agm = ag[s0] + ssel * (ag[s1] - ag[s0])
        d8 = wp.tile([128, 8], f32)
        nc.vector.tensor_sub(d8, ag[:, 8:16], ag[:, 0:8])
        agm = wp.tile([128, 8], f32)
        nc.vector.scalar_tensor_tensor(
            agm, d8, ssel[:, 0:1], ag[:, 0:8], op0=ALU.mult, op1=ALU.add)

        # a = gamma / sqrt(var+eps), c = beta - mu * a
        mu = wp.tile([128, NDC], f32)
        nc.vector.tensor_scalar_mul(mu, agm[:, 0:4], inv_n)
        var = wp.tile([128, NDC], f32)
        nc.vector.tensor_scalar_mul(var, agm[:, 4:8], inv_n)
        musq = wp.tile([128, NDC], f32)
        nc.vector.tensor_mul(musq, mu, mu)
        nc.vector.tensor_sub(var, var, musq)
        nc.vector.tensor_scalar_add(var, var, EPS)
        sig = wp.tile([128, NDC], f32)
        nc.scalar.activation(sig, var, AF.Sqrt, bias=eps_t[:, 0:1])
        isig = wp.tile([128, NDC], f32)
        nc.vector.reciprocal(isig, sig)
        bn_a = wp.tile([128, NDC], f32)
        nc.vector.tensor_mul(bn_a, gamS, isig)
        bn_c = wp.tile([128, NDC], f32)
        nc.vector.tensor_mul(bn_c, mu, bn_a)
        nc.vector.tensor_sub(bn_c, betS, bn_c)

        # ---------------- Phase B: recurrence (one stream, 32 batch) ------
        pb = stk.enter_context(tc.tile_pool(name="pb", bufs=2))
        ps = stk.enter_context(tc.tile_pool(name="ps", bufs=2, space="PSUM"))

        h_zero = wp.tile([128, NDC, BSP], f16)
        nc.vector.memset(h_zero, 0.0)
        c_state = wp.tile([128, NDC, BSP], f32)
        nc.vector.memset(c_state, 0.0)
        h_prev = h_zero

        def load_z(c):
            t0 = c * TC
            zc = pb.tile([128, NDC, BSP, TC], f16, tag="zc")
            for dc in range(NDC):
                nc.gpsimd.dma_start(
                    out=zc[:, dc],
                    in_=zin[dc].rearrange("p (b t) -> p b t", b=BSP)[
                        :, :, t0:t0 + TC])
            return zc

        def make_e(zc):
            e = pb.tile([128, NDC, BSP, TC], f16, tag="e")
            for dc in range(NDC):
                nc.scalar.activation(
                    e[:, dc], zc[:, dc], AF.Relu,
                    bias=bn_c[:, dc:dc + 1],
                    scale=bn_a[:, dc:dc + 1])
            return e

        def gating_units(c, hga):
            """Return a list of closures that together perform gating/fusion/
            output for chunk c. Issued interleaved between recurrence steps
            of chunk c+1 so their PE work fills the per-step dependency gaps
            (engine queues are in-order)."""
            units = []
            st = {}
            t0 = c * TC

            def mk_sel(s):
                def f():
                    hd = pb.tile([128, TC, NDC, BS], f16, tag=f"hd{s}",
                                 bufs=1)
                    nc.vector.tensor_tensor(
                        hd, hga[:, s, :, :, BS:BSP], hga[:, s, :, :, 0:BS],
                        op=ALU.subtract)
                    hs = pb.tile([128, TC, NDC, BS], f16, tag=f"hs{s}")
                    nc.vector.scalar_tensor_tensor(
                        hs, hd, ssel[:, 0:1], hga[:, s, :, :, 0:BS],
                        op0=ALU.mult, op1=ALU.add)
                    st[f"hs{s}"] = hs
                return f
            units.append(mk_sel(0))
            units.append(mk_sel(1))

            def mk_pg_alloc(s):
                def f():
                    st[f"pg{s}"] = ps.tile([128, NDC, BS, TC], f32,
                                           tag="lag", name=f"pg{s}",
                                           bufs=1)
                return f

            def mk_gate_mm(s, go):
                def f():
                    src = st[f"hs{1 - s}"]
                    wgT = (wg0, wg1)[s]
                    bgt = (bg0, bg1)[s]
                    pg = st[f"pg{s}"]
                    for dc in range(NDC):
                        nc.tensor.matmul(
                            pg[:, go],
                            lhsT=wgT[:, dc, go * 128:(go + 1) * 128],
                            rhs=src[:, :, dc, :].rearrange("p t b -> p b t"),
                            start=(dc == 0), stop=False)
                    nc.tensor.matmul(
                        pg[:, go], lhsT=bgt[0:1, go * 128:(go + 1) * 128],
                        rhs=ones16[0:1, 0:BS * TC],
                        start=False, stop=True)
                return f

            def mk_gate_fin(s):
                def f():
                    sp = pb.tile([128, NDC, BS, TC], f16, tag="sp")
                    nc.scalar.activation(sp, st[f"pg{s}"], AF.Sigmoid)
                    ot = pb.tile([128, NDC, BS, TC], f16, tag=f"o{s}")
                    nc.vector.tensor_mul(
                        ot, sp, st[f"hs{s}"].rearrange("p t c b -> p c b t"))
                    st[f"o{s}"] = ot
                return f

            for s in range(2):
                units.append(mk_pg_alloc(s))
                for go in range(NDC):
                    units.append(mk_gate_mm(s, go))
                units.append(mk_gate_fin(s))

            def fp_alloc():
                st["fp"] = ps.tile([128, NDC, BS, TC], f32, tag="lag",
                                   name="fp", bufs=1)
            units.append(fp_alloc)

            def mk_fusion(do):
                def f():
                    fp_ = st["fp"]
                    first = True
                    for s, wfT in enumerate((wf1, wf2)):
                        for dc in range(NDC):
                            nc.tensor.matmul(
                                fp_[:, do],
                                lhsT=wfT[:, dc, do * 128:(do + 1) * 128],
                                rhs=st[f"o{s}"][:, dc],
                                start=first, stop=False)
                            first = False
                    nc.tensor.matmul(
                        fp_[:, do], lhsT=bf[0:1, do * 128:(do + 1) * 128],
                        rhs=ones16[0:1, 0:BS * TC],
                        start=False, stop=True)
                return f
            for do in range(NDC):
                units.append(mk_fusion(do))

            def otn_act():
                otn = pb.tile([128, NDC, BS, TC], f16, tag="otn")
                nc.scalar.activation(otn, st["fp"], AF.Tanh)
                st["otn"] = otn
            units.append(otn_act)

            def mk_store(bh):
                def f():
                    on = pb.tile([128, NDC, 128], f32, tag="on")
                    for do in range(NDC):
                        tp2 = ps.tile([128, 128], f16, tag="gif",
                                      bufs=1)
                        nc.tensor.transpose(
                            tp2, st["otn"][:, do, bh * 8:(bh + 1) * 8, :],
                            ident)
                        nc.vector.tensor_copy(on[:, do], tp2)
                    nc.gpsimd.dma_start(
                        out=out_d.rearrange("b t (c p) -> b t c p", p=128)[
                            bh * 8:(bh + 1) * 8, t0:t0 + TC],
                        in_=on)
                return f
            units.append(mk_store(0))
            units.append(mk_store(1))
            return units

        def gating(c, hga):
            t0 = c * TC
            # select my 16-batch half of both streams
            hsel = []
            for s in range(2):
                hd = pb.tile([128, TC, NDC, BS], f16, tag=f"hd{s}", bufs=1)
                nc.vector.tensor_tensor(
                    hd, hga[:, s, :, :, BS:BSP], hga[:, s, :, :, 0:BS],
                    op=ALU.subtract)
                hs = pb.tile([128, TC, NDC, BS], f16, tag=f"hs{s}")
                nc.vector.scalar_tensor_tensor(
                    hs, hd, ssel[:, 0:1], hga[:, s, :, :, 0:BS],
                    op0=ALU.mult, op1=ALU.add)
                hsel.append(hs)

            o_t = []
            for s in range(2):
                src = hsel[1 - s]  # gate for stream s reads the OTHER h
                wgT = (wg0, wg1)[s]
                bgt = (bg0, bg1)[s]
                pg = ps.tile([128, NDC, BS, TC], f32, tag="lag",
                             bufs=1)
                for go in range(NDC):
                    for dc in range(NDC):
                        nc.tensor.matmul(
                            pg[:, go],
                            lhsT=wgT[:, dc, go * 128:(go + 1) * 128],
                            rhs=src[:, :, dc, :].rearrange("p t b -> p b t"),
                            start=(dc == 0), stop=False)
                    nc.tensor.matmul(
                        pg[:, go], lhsT=bgt[0:1, go * 128:(go + 1) * 128],
                        rhs=ones16[0:1, 0:BS * TC],
                        start=False, stop=True)
                sp = pb.tile([128, NDC, BS, TC], f16, tag="sp")
                nc.scalar.activation(sp, pg, AF.Sigmoid)
                ot = pb.tile([128, NDC, BS, TC], f16, tag=f"o{s}")
                nc.vector.tensor_mul(
                    ot, sp, hsel[s].rearrange("p t c b -> p c b t"))
                o_t.append(ot)

            # fusion: tanh(wf1.T @ o0 + wf2.T @ o1 + bf)
            fp_ = ps.tile([128, NDC, BS, TC], f32, tag="lag",
                          bufs=1)
            for do in range(NDC):
                first = True
                for s, wfT in enumerate((wf1, wf2)):
                    for dc in range(NDC):
                        nc.tensor.matmul(
                            fp_[:, do],
                            lhsT=wfT[:, dc, do * 128:(do + 1) * 128],
                            rhs=o_t[s][:, dc],
                            start=first, stop=False)
                        first = False
                nc.tensor.matmul(
                    fp_[:, do], lhsT=bf[0:1, do * 128:(do + 1) * 128],
                    rhs=ones16[0:1, 0:BS * TC],
                    start=False, stop=True)
            otn = pb.tile([128, NDC, BS, TC], f16, tag="otn")
            nc.scalar.activation(otn, fp_, AF.Tanh)

            # transpose back to natural layout and store
            for bh in range(2):
                on = pb.tile([128, NDC, 128], f32, tag="on")
                for do in range(NDC):
                    tp2 = ps.tile([128, 128], f16, tag="gif",
                                  bufs=1)
                    nc.tensor.transpose(
                        tp2, otn[:, do, bh * 8:(bh + 1) * 8, :], ident)
                    nc.vector.tensor_copy(on[:, do], tp2)
                nc.gpsimd.dma_start(
                    out=out_d.rearrange("b t (c p) -> b t c p", p=128)[
                        bh * 8:(bh + 1) * 8, t0:t0 + TC],
                    in_=on)

        zc_cur = load_z(0)
        e_cur = make_e(zc_cur)
        hga_prev = None
        for c in range(nchunk):
            t0 = c * TC
            if c + 1 < nchunk:
                zc_next = load_z(c + 1)

            # -- input projections u = e @ w_ih.T + bc --
            ut = pb.tile([128, NGC, BSP, TC], f16, tag="u")
            # produce u in step-consumption order: g-gate blocks first so
            # the first steps can start while the rest of u is computed
            for g in (list(range(3 * NDC, 4 * NDC)) + list(range(0, 3 * NDC))):
                up = ps.tile([128, BSP, TC], f32, tag="u")
                for dc in range(NDC):
                    nc.tensor.matmul(
                        up,
                        lhsT=wih[:, dc, g * 128:(g + 1) * 128],
                        rhs=e_cur[:, dc],
                        start=(dc == 0), stop=(dc == NDC - 1))
                nc.scalar.activation(
                    ut[:, g], up, AF.Identity, bias=bcc[:, g:g + 1])

            # -- mask broadcast for this chunk (two 16-batch halves) --
            msk = pb.tile([128, NDC, BSP, TC], f16, tag="msk")
            for g in range(2):
                mp = ps.tile([128, NDC, BS, TC], f32, tag="lag",
                             bufs=1)
                for dc in range(NDC):
                    nc.tensor.matmul(
                        mp[:, dc],
                        lhsT=ones16[0:1, 0:128],
                        rhs=mflat[0:1, g * BS:(g + 1) * BS, t0:t0 + TC],
                        start=True, stop=True)
                nc.vector.tensor_copy(msk[:, :, g * BS:(g + 1) * BS], mp)

            # -- recurrence steps (with chunk c-1's gating interleaved
            #    into the per-step dependency gaps) --
            units = gating_units(c - 1, hga_prev) if hga_prev is not None \
                else []
            udone = 0
            hh_t = pb.tile([128, TC, NDC, BSP], f16, tag="hh", bufs=3,
                           name="hh")
            for tl in range(TC):
                m_sl = msk[:, :, :, tl]
                # separate psum tiles per gate group so each group's
                # +u add / activation can run while later groups' matmuls
                # still write their own psum (no tile-level WAR)
                gp_g = ps.tile([128, NDC, BSP], f32, tag="gg", bufs=1)
                gp_if = ps.tile([128, 2 * NDC, BSP], f32, tag="gif", bufs=1)
                gp_o = ps.tile([128, NDC, BSP], f32, tag="go", bufs=1)

                def mmgrp(dst, glo, ghi):
                    for g in range(glo, ghi):
                        for dc in range(NDC):
                            nc.tensor.matmul(
                                dst[:, g - glo],
                                lhsT=whh[:, dc, g * 128:(g + 1) * 128],
                                rhs=h_prev[:, dc],
                                start=(dc == 0), stop=(dc == NDC - 1))
                mmgrp(gp_g, 3 * NDC, 4 * NDC)     # g-gate first
                gsb_g = pb.tile([128, NDC, BSP], f32, tag="gsbg")
                nc.vector.tensor_tensor(
                    gsb_g, gp_g, ut[:, 3 * NDC:4 * NDC, :, tl], op=ALU.add)
                tg = pb.tile([128, NDC, BSP], f32, tag="tg")
                nc.scalar.activation(tg, gsb_g, AF.Tanh)
                mmgrp(gp_if, 0, 2 * NDC)          # i, f
                gsb_if = pb.tile([128, 2 * NDC, BSP], f32, tag="gsbif")
                nc.vector.tensor_tensor(
                    gsb_if, gp_if, ut[:, 0:2 * NDC, :, tl], op=ALU.add)
                sgif = pb.tile([128, 2 * NDC, BSP], f32, tag="sgif")
                nc.scalar.activation(sgif, gsb_if, AF.Sigmoid)
                t1 = pb.tile([128, NDC, BSP], f32, tag="t1")
                nc.vector.tensor_mul(t1, sgif[:, 0:NDC], tg)
                t2 = pb.tile([128, NDC, BSP], f32, tag="t2")
                nc.vector.tensor_mul(t2, sgif[:, NDC:2 * NDC], c_state)
                mmgrp(gp_o, 2 * NDC, 3 * NDC)     # o last
                gsb_o = pb.tile([128, NDC, BSP], f32, tag="gsbo")
                nc.vector.tensor_tensor(
                    gsb_o, gp_o, ut[:, 2 * NDC:3 * NDC, :, tl], op=ALU.add)
                cn = pb.tile([128, NDC, BSP], f32, tag="cn")
                nc.vector.tensor_add(cn, t1, t2)
                sgo = pb.tile([128, NDC, BSP], f32, tag="sgo")
                nc.scalar.activation(sgo, gsb_o, AF.Sigmoid)
                th = pb.tile([128, NDC, BSP], f32, tag="th")
                nc.scalar.activation(th, cn, AF.Tanh)
                om = pb.tile([128, NDC, BSP], f32, tag="om")
                nc.vector.tensor_mul(om, sgo, m_sl)
                nc.vector.tensor_mul(hh_t[:, tl], om, th)
                nc.vector.tensor_mul(c_state, cn, m_sl)
                h_prev = hh_t[:, tl]
                # interleave lagged gating work (skip step 0: give the
                # h AllGather of chunk c-1 time to land)
                if tl >= 1:
                    target = (tl * len(units)) // (TC - 1)
                    while udone < target:
                        units[udone]()
                        udone += 1

            # -- BN+ReLU for the next chunk (z arrived during the steps) --
            if c + 1 < nchunk:
                e_cur = make_e(zc_next)

            # -- pair h exchange for this chunk --
            db = c % 2
            nc.sync.dma_start(out=hin_d[db][:, :, :, :], in_=hh_t)
            if use_collective:
                nc.gpsimd.collective_compute(
                    "AllGather", ALU.bypass,
                    replica_groups=PAIRS,
                    ins=[hin_d[db][:, :, :, :]],
                    outs=[hga_d[db][:, :, :, :, :]])
            else:
                nc.sync.dma_start(out=hga_d[db][0], in_=hin_d[db][:, :, :, :])
                nc.sync.dma_start(out=hga_d[db][1], in_=hin_d[db][:, :, :, :])
            hga = pb.tile([128, 2, TC, NDC, BSP], f16, tag="hga")
            for s in range(2):
                nc.sync.dma_start(out=hga[:, s], in_=hga_d[db][s])

            # -- finish any remaining gating units for chunk c-1 --
            while udone < len(units):
                units[udone]()
                udone += 1
            hga_prev = hga

        gating(nchunk - 1, hga_prev)

    nc.compile()
    return nc


def _prep_weights(i):
    """Host-side weight packing: fp16 casts, transposes, gate reorder.
    Returns (shared, per_stream[2]) dicts."""
    def perm_gates_rows(w):  # [4D, ...] rows (i,f,g,o) -> (i,f,o,g)
        return np.concatenate(
            [w[0:D], w[D:2 * D], w[3 * D:4 * D], w[2 * D:3 * D]], axis=0)

    f16 = np.float16
    shared = {}
    for s in range(2):
        shared[f"wg{s}T"] = np.ascontiguousarray(i[f"wg{s}"].T.astype(f16))
        shared[f"bg{s}"] = i[f"bg{s}"].astype(f16).reshape(1, D)
    shared["wf1T"] = np.ascontiguousarray(i["wf1"].T.astype(f16))
    shared["wf2T"] = np.ascontiguousarray(i["wf2"].T.astype(f16))
    shared["bf"] = i["bf"].astype(f16).reshape(1, D)

    per_stream = []
    for s in range(2):
        d = {}
        we = i[f"w_emb{s}"].T.astype(f16)           # [Fs, D]
        if we.shape[0] < FR:
            we = np.vstack([we, np.zeros((FR - we.shape[0], D), f16)])
        d["w_embST"] = np.ascontiguousarray(we)
        d["w_ihST"] = np.ascontiguousarray(
            perm_gates_rows(i[f"w_ih{s}"]).T.astype(f16))
        d["w_hhST"] = np.ascontiguousarray(
            perm_gates_rows(i[f"w_hh{s}"]).T.astype(f16))
        d["bcS"] = perm_gates_rows(
            (i[f"b_ih{s}"] + i[f"b_hh{s}"]).reshape(4 * D, 1))[:, 0].astype(
                np.float32)
        d["gammaS"] = i[f"gamma{s}"].astype(np.float32)
        d["betaS"] = i[f"beta{s}"].astype(np.float32)
        d["ssel"] = np.full(128, float(s), np.float32)
        d["nssel"] = np.full(128, 1.0 - float(s), np.float32)
        per_stream.append(d)
    return shared, per_stream


def _make_in_maps(inputs):
    shared, per_stream = _prep_weights(inputs)
    feats = (inputs["feat0"], inputs["feat1"])
    in_maps = []
    for cid in range(NCORES):
        parity = cid % 2
        pair = cid // 2
        m = dict(shared)
        m.update(per_stream[parity])
        psl = slice(pair * BSP, (pair + 1) * BSP)
        f = np.asarray(feats[parity][psl], np.float32).reshape(BSP * T, -1)
        if f.shape[1] < FR:
            f = np.concatenate(
                [f, np.zeros((f.shape[0], FR - f.shape[1]), np.float32)],
                axis=1)
        m["featS"] = np.ascontiguousarray(f)
        m["feat_mask"] = np.ascontiguousarray(
            inputs["feat_mask"][psl].astype(np.float32))
        in_maps.append(m)
    return in_maps


def kernel(**inputs):
    from concourse.bass_utils import run_bass_kernel_spmd

    global _BUILT
    if _BUILT is None:
        _BUILT = _build(T)
    nc = _BUILT

    in_maps = _make_in_maps(inputs)
    res = run_bass_kernel_spmd(nc, in_maps, core_ids=list(range(NCORES)))
    outs = [res.results[cid]["out"] for cid in range(NCORES)]
    return np.concatenate(outs, axis=0)


if __name__ == "__main__":
    nc = _build(T)
    print("built ok")



# revision 8
# speedup vs baseline: 1.3774x; 1.3774x over previous
"""Trainium2 Bass kernel for the two-stream LSTM encoder.

Strategy (8 NeuronCores): cores are paired; each core runs ONE LSTM stream
for its pair's 32 batch rows (halves recurrent weight streaming). SPMD: the
program is identical on all cores; stream identity enters only through
per-core input tensors (ssel selects drive the few data-dependent spots).

  - Features are uploaded pre-transposed fp16 [F, T*BSP] (t-major columns)
    so Phase A is a straight matmul with no on-device transposes or casts.
  - Phase A: z = w_embS.T-style matmul into T-layout, spilled to DRAM fp16;
    BN stats all-reduced across the 8 cores into per-stream slots.
  - Phase B: time loop in chunks of 16 steps. Per step, the input
    projection u (precomputed per chunk) is injected into PSUM via an
    identity matmul so the recurrent matmuls accumulate on top of it —
    the post-matmul serial chain is just sigmoid(o) -> h mul. The mask is
    identically 1.0 (per the model's input spec) and is elided. Gate-group
    PSUM tiles are bank-padded to avoid read/write bank serialization.
  - Next chunk's u-projection and previous chunk's gating/fusion are
    interleaved into the per-step dependency gaps; gating elementwise runs
    on GpSimd so it never queues ahead of the critical-path DVE/ACT ops.
  - Output is stored in a raw chunk-major layout and reassembled on host.
Gate order is host-permuted from torch's (i,f,g,o) to (i,f,o,g).
"""

import os
import numpy as np

os.environ.setdefault("MYCRO_LOCAL_CACHE", "1")

NCORES = 8
B, T, D, F0, F1 = 128, 256, 512, 2048, 1024
FR = 2048                 # padded feature width (shared by both streams)
BS = B // NCORES          # 16 batch rows per core for output
BSP = 2 * BS              # 32 batch rows per pair (recurrence batch)
TC = 16                   # time steps per chunk
NCHUNK = T // TC
EPS = 1e-5
G4 = 4 * D                # 2048 gate dim
NDC = D // 128            # 4 d-chunks
NGC = G4 // 128           # 16 gate chunks
NFC = FR // 128           # 16 feature chunks

PAIRS = [[2 * i, 2 * i + 1] for i in range(NCORES // 2)]

_BUILT = None


def _build(t_steps=T, use_collective=True):
    import concourse.bass as bass
    import concourse.bacc as bacc
    import concourse.mybir as mybir
    import concourse.tile as tile
    from concourse.masks import make_identity
    from contextlib import ExitStack

    f16 = mybir.dt.float16
    f32 = mybir.dt.float32
    AF = mybir.ActivationFunctionType
    ALU = mybir.AluOpType

    nchunk = t_steps // TC
    rows2 = BSP * t_steps     # 8192 pair rows (t-major: r = t*BSP + b)
    nrc2 = rows2 // 512       # 16 row chunks of 512

    nc = bacc.Bacc(None, num_devices=NCORES)

    # ---------------- DRAM parameters ----------------
    featS = nc.declare_dram_parameter("featST", [FR, rows2], f16,
                                      isOutput=False)
    wembp = nc.declare_dram_parameter("w_embST", [FR, D], f16, isOutput=False)
    wihp = nc.declare_dram_parameter("w_ihST", [D, G4], f16, isOutput=False)
    whhp = nc.declare_dram_parameter("w_hhST", [D, G4], f16, isOutput=False)
    wg0T = nc.declare_dram_parameter("wg0T", [D, D], f16, isOutput=False)
    wg1T = nc.declare_dram_parameter("wg1T", [D, D], f16, isOutput=False)
    wf1T = nc.declare_dram_parameter("wf1T", [D, D], f16, isOutput=False)
    wf2T = nc.declare_dram_parameter("wf2T", [D, D], f16, isOutput=False)
    bcp = nc.declare_dram_parameter("bcS", [G4], f32, isOutput=False)
    bg0p = nc.declare_dram_parameter("bg0c", [D], f32, isOutput=False)
    bg1p = nc.declare_dram_parameter("bg1c", [D], f32, isOutput=False)
    bfp = nc.declare_dram_parameter("bfc", [D], f32, isOutput=False)
    gammap = nc.declare_dram_parameter("gammaS", [D], f32, isOutput=False)
    betap = nc.declare_dram_parameter("betaS", [D], f32, isOutput=False)
    sselp = nc.declare_dram_parameter("ssel", [128], f32, isOutput=False)
    nsselp = nc.declare_dram_parameter("nssel", [128], f32, isOutput=False)
    out_d = nc.declare_dram_parameter(
        "out", [nchunk, 128, NDC, TC, BS], f16, isOutput=True)

    # DRAM scratch
    zin = nc.dram_tensor("zin", [NDC, 128, rows2], f16)
    st_in = nc.dram_tensor("st_in", [128, 16], f32)
    st_out = nc.dram_tensor("st_out", [128, 16], f32)
    hin_d = [nc.dram_tensor(f"hin{i}", [128, TC, NDC, BSP], f16)
             for i in range(2)]
    hga_d = [nc.dram_tensor(f"hga{i}", [2, 128, TC, NDC, BSP], f16)
             for i in range(2)]

    inv_n = 1.0 / float(B * t_steps)
    if not use_collective:
        inv_n = 1.0 / float(BSP * t_steps)

    with tile.TileContext(nc) as tc, ExitStack() as stk:
        wp = stk.enter_context(tc.tile_pool(name="wp", bufs=1))

        # ---------------- resident small tiles ----------------
        ident = wp.tile([128, 128], f16)
        make_identity(nc, ident)
        eps_t = wp.tile([128, 1], f32)
        nc.vector.memset(eps_t, 0.0)
        ssel = wp.tile([128, 1], f32)
        nc.sync.dma_start(out=ssel, in_=sselp.rearrange("(p o) -> p o", o=1))
        nssel = wp.tile([128, 1], f32)
        nc.sync.dma_start(out=nssel, in_=nsselp.rearrange("(p o) -> p o", o=1))

        def load_T(dram, nch, free):
            t = wp.tile([128, nch, free], f16, name=f"w_{dram.name}")
            nc.sync.dma_start(
                out=t, in_=dram.rearrange("(c p) g -> p c g", p=128))
            return t

        wih = load_T(wihp, NDC, G4)
        whh = load_T(whhp, NDC, G4)
        wg0 = load_T(wg0T, NDC, D)
        wg1 = load_T(wg1T, NDC, D)
        wf1 = load_T(wf1T, NDC, D)
        wf2 = load_T(wf2T, NDC, D)

        def load_col(dram, nch):
            t = wp.tile([128, nch], f32, name=f"b_{dram.name}")
            nc.sync.dma_start(
                out=t, in_=dram.rearrange("(g p) -> p g", p=128))
            return t

        bcc = load_col(bcp, NGC)     # cell bias, (i,f,o,g) column layout
        bgc = [load_col(bg0p, NDC), load_col(bg1p, NDC)]
        bfc = load_col(bfp, NDC)
        gamS = load_col(gammap, NDC)
        betS = load_col(betap, NDC)

        # per-rowgroup BN stat slots for MY stream: col = dc*nrc2 + rc
        stat_s = wp.tile([128, NDC * nrc2], f32, name="stat_s")
        stat_q = wp.tile([128, NDC * nrc2], f32, name="stat_q")

        # ---------------- Phase A: embed my stream (t-major cols) ---------
        with tc.tile_pool(name="pa", bufs=2) as pa, \
             tc.tile_pool(name="paw", bufs=1) as paw, \
             tc.tile_pool(name="psA", bufs=1, space="PSUM") as psA:
            wemb = paw.tile([128, NFC, D], f16)
            nc.sync.dma_start(
                out=wemb, in_=wembp.rearrange("(c p) g -> p c g", p=128))
            for rc in range(nrc2):
                fT = pa.tile([128, NFC, 512], f16, tag="fT")
                nc.gpsimd.dma_start(
                    out=fT,
                    in_=featS.rearrange("(c p) r -> p c r", p=128)[
                        :, :, rc * 512:(rc + 1) * 512])
                za = psA.tile([128, NDC, 512], f32, tag="za", bufs=2)
                for dc in range(NDC):
                    for fc in range(NFC):
                        nc.tensor.matmul(
                            za[:, dc],
                            lhsT=wemb[:, fc, dc * 128:(dc + 1) * 128],
                            rhs=fT[:, fc],
                            start=(fc == 0), stop=(fc == NFC - 1))
                zst = pa.tile([128, NDC, 512], f16, tag="zst")
                sq = pa.tile([128, 512], f32, tag="sq")
                for dc in range(NDC):
                    nc.scalar.activation(
                        zst[:, dc], za[:, dc],
                        AF.Identity,
                        accum_out=stat_s[:, dc * nrc2 + rc:
                                         dc * nrc2 + rc + 1])
                    nc.vector.tensor_tensor(
                        sq, za[:, dc], zst[:, dc], op=ALU.mult)
                    nc.vector.reduce_sum(
                        stat_q[:, dc * nrc2 + rc:dc * nrc2 + rc + 1],
                        sq, axis=mybir.AxisListType.X)
                nc.gpsimd.dma_start(
                    out=zin[:, :, rc * 512:(rc + 1) * 512].rearrange(
                        "c p r -> p c r"),
                    in_=zst)

        # ---------------- BN stats allreduce (per-stream slots) -----------
        ared = wp.tile([128, 8], f32)
        nc.vector.reduce_sum(
            ared[:, 0:4],
            stat_s.rearrange("p (c r) -> p c r", c=NDC),
            axis=mybir.AxisListType.X)
        nc.vector.reduce_sum(
            ared[:, 4:8],
            stat_q.rearrange("p (c r) -> p c r", c=NDC),
            axis=mybir.AxisListType.X)
        # slot 0:8 <- stream0 contribution, 8:16 <- stream1 (ssel-masked)
        sts = wp.tile([128, 16], f32)
        nc.scalar.activation(sts[:, 0:8], ared, AF.Identity,
                             scale=nssel[:, 0:1])
        nc.scalar.activation(sts[:, 8:16], ared, AF.Identity,
                             scale=ssel[:, 0:1])
        nc.gpsimd.dma_start(out=st_in[:, :], in_=sts)
        if use_collective:
            nc.gpsimd.collective_compute(
                "AllReduce", ALU.add,
                replica_groups=[list(range(NCORES))],
                ins=[st_in[:, :]], outs=[st_out[:, :]])
        else:
            nc.gpsimd.dma_start(out=st_out[:, :], in_=st_in[:, :])
        ag = wp.tile([128, 16], f32)
        nc.gpsimd.dma_start(out=ag, in_=st_out[:, :])

        # my stream's stats: agm = ag[s0] + ssel * (ag[s1] - ag[s0])
        d8 = wp.tile([128, 8], f32)
        nc.vector.tensor_sub(d8, ag[:, 8:16], ag[:, 0:8])
        agm = wp.tile([128, 8], f32)
        nc.vector.scalar_tensor_tensor(
            agm, d8, ssel[:, 0:1], ag[:, 0:8], op0=ALU.mult, op1=ALU.add)

        # a = gamma / sqrt(var+eps), c = beta - mu * a
        mu = wp.tile([128, NDC], f32)
        nc.vector.tensor_scalar_mul(mu, agm[:, 0:4], inv_n)
        var = wp.tile([128, NDC], f32)
        nc.vector.tensor_scalar_mul(var, agm[:, 4:8], inv_n)
        musq = wp.tile([128, NDC], f32)
        nc.vector.tensor_mul(musq, mu, mu)
        nc.vector.tensor_sub(var, var, musq)
        nc.vector.tensor_scalar_add(var, var, EPS)
        sig = wp.tile([128, NDC], f32)
        nc.scalar.activation(sig, var, AF.Sqrt, bias=eps_t[:, 0:1])
        isig = wp.tile([128, NDC], f32)
        nc.vector.reciprocal(isig, sig)
        bn_a = wp.tile([128, NDC], f32)
        nc.vector.tensor_mul(bn_a, gamS, isig)
        bn_c = wp.tile([128, NDC], f32)
        nc.vector.tensor_mul(bn_c, mu, bn_a)
        nc.vector.tensor_sub(bn_c, betS, bn_c)

        # ---------------- Phase B: recurrence (one stream, 32 batch) ------
        pb = stk.enter_context(tc.tile_pool(name="pb", bufs=2))
        ps = stk.enter_context(tc.tile_pool(name="ps", bufs=2, space="PSUM"))

        h_zero = wp.tile([128, NDC, BSP], f16)
        nc.vector.memset(h_zero, 0.0)
        # c-state ping-pong (mask==1 so no separate masked copy is needed)
        c_ab = [wp.tile([128, NDC, BSP], f32, name=f"c{i}") for i in range(2)]
        nc.vector.memset(c_ab[0], 0.0)
        nc.vector.memset(c_ab[1], 0.0)
        # u double-buffer by chunk parity: [128, NGC, TC, BSP] fp16
        ut2 = [wp.tile([128, NGC, TC, BSP], f16, name=f"ut{i}")
               for i in range(2)]

        UPROJ_ORDER = list(range(3 * NDC, 4 * NDC)) + list(range(0, 3 * NDC))

        def load_z(c):
            zc = pb.tile([128, NDC, TC, BSP], f16, tag="zc")
            nc.gpsimd.dma_start(
                out=zc,
                in_=zin.rearrange("c p r -> p c r")[
                    :, :, c * TC * BSP:(c + 1) * TC * BSP])
            return zc

        def make_e(zc):
            e = pb.tile([128, NDC, TC, BSP], f16, tag="e")
            for dc in range(NDC):
                nc.scalar.activation(
                    e[:, dc], zc[:, dc], AF.Relu,
                    bias=bn_c[:, dc:dc + 1],
                    scale=bn_a[:, dc:dc + 1])
            return e

        def uproj_units(e_src, pc):
            """u = e @ w_ih.T + bc for one chunk into ut2[pc] (16 units)."""
            units = []

            def mk(g):
                def f():
                    up = ps.tile([128, TC, BSP], f32, tag="u", bufs=2)
                    for dc in range(NDC):
                        nc.tensor.matmul(
                            up,
                            lhsT=wih[:, dc, g * 128:(g + 1) * 128],
                            rhs=e_src[:, dc],
                            start=(dc == 0), stop=(dc == NDC - 1))
                    nc.scalar.activation(
                        ut2[pc][:, g], up, AF.Identity, bias=bcc[:, g:g + 1])
                return f
            for g in UPROJ_ORDER:
                units.append(mk(g))
            return units

        def gating_units(c, hga):
            """Gating/fusion/output for chunk c (uses chunk c's h, gathered
            from the pair). All elementwise is split per-dc (~256 cols per
            op) so no unit can queue-delay the per-step critical-path
            DVE/ACT ops by more than ~0.2us."""
            units = []
            st = {}

            def mk_sel_alloc(s):
                def f():
                    st[f"hd{s}"] = pb.tile([128, TC, NDC, BS], f16,
                                           tag=f"hd{s}", bufs=1,
                                           name=f"hd{s}")
                    st[f"hs{s}"] = pb.tile([128, TC, NDC, BS], f16,
                                           tag=f"hs{s}", name=f"hs{s}")
                return f

            def mk_sel(s, dc):
                def f():
                    hd = st[f"hd{s}"]
                    hs = st[f"hs{s}"]
                    nc.vector.tensor_tensor(
                        hd[:, :, dc], hga[:, s, :, dc, BS:BSP],
                        hga[:, s, :, dc, 0:BS], op=ALU.subtract)
                    nc.vector.scalar_tensor_tensor(
                        hs[:, :, dc], hd[:, :, dc], ssel[:, 0:1],
                        hga[:, s, :, dc, 0:BS],
                        op0=ALU.mult, op1=ALU.add)
                return f
            for s in range(2):
                units.append(mk_sel_alloc(s))
                for dc in range(NDC):
                    units.append(mk_sel(s, dc))

            def mk_pg_alloc(s):
                def f():
                    st[f"pg{s}"] = ps.tile([128, NDC, TC, BS], f32,
                                           tag="lag", name=f"pg{s}",
                                           bufs=1)
                return f

            def mk_gate_mm(s, go):
                def f():
                    src = st[f"hs{1 - s}"]
                    wgT = (wg0, wg1)[s]
                    pg = st[f"pg{s}"]
                    for dc in range(NDC):
                        nc.tensor.matmul(
                            pg[:, go],
                            lhsT=wgT[:, dc, go * 128:(go + 1) * 128],
                            rhs=src[:, :, dc, :],
                            start=(dc == 0), stop=(dc == NDC - 1))
                return f

            def mk_fin_alloc(s):
                def f():
                    st[f"sp{s}"] = pb.tile([128, NDC, TC, BS], f16,
                                           tag=f"sp{s}", name=f"sp{s}")
                    st[f"o{s}"] = pb.tile([128, NDC, TC, BS], f16,
                                          tag=f"o{s}", name=f"o{s}")
                return f

            def mk_gate_fin(s, dc):
                def f():
                    sp = st[f"sp{s}"]
                    nc.scalar.activation(
                        sp[:, dc], st[f"pg{s}"][:, dc], AF.Sigmoid,
                        bias=bgc[s][:, dc:dc + 1])
                    nc.vector.tensor_tensor(
                        st[f"o{s}"][:, dc], sp[:, dc],
                        st[f"hs{s}"][:, :, dc], op=ALU.mult)
                return f

            for s in range(2):
                units.append(mk_pg_alloc(s))
                for go in range(NDC):
                    units.append(mk_gate_mm(s, go))
                units.append(mk_fin_alloc(s))
                for dc in range(NDC):
                    units.append(mk_gate_fin(s, dc))

            def fp_alloc():
                st["fp"] = ps.tile([128, NDC, TC, BS], f32, tag="lag",
                                   name="fp", bufs=1)
            units.append(fp_alloc)

            def mk_fusion(do, s):
                def f():
                    wfT = (wf1, wf2)[s]
                    for dc in range(NDC):
                        nc.tensor.matmul(
                            st["fp"][:, do],
                            lhsT=wfT[:, dc, do * 128:(do + 1) * 128],
                            rhs=st[f"o{s}"][:, dc],
                            start=(s == 0 and dc == 0),
                            stop=(s == 1 and dc == NDC - 1))
                return f
            for do in range(NDC):
                for s in range(2):
                    units.append(mk_fusion(do, s))

            def otn_alloc():
                st["otn"] = pb.tile([128, NDC, TC, BS], f16, tag="otn",
                                    name="otn")
            units.append(otn_alloc)

            def mk_otn(dc):
                def f():
                    nc.scalar.activation(
                        st["otn"][:, dc], st["fp"][:, dc], AF.Tanh,
                        bias=bfc[:, dc:dc + 1])
                return f
            for dc in range(NDC):
                units.append(mk_otn(dc))

            def store():
                nc.gpsimd.dma_start(out=out_d[c], in_=st["otn"])
            units.append(store)
            return units

        # ---- chunk 0 prologue: z, e, u (dense) ----
        zc_cur = load_z(0)
        e_cur = make_e(zc_cur)
        for u in uproj_units(e_cur, 0):
            u()

        hga_prev = None
        h_prev = h_zero
        for c in range(nchunk):
            units = []
            if hga_prev is not None:
                units += gating_units(c - 1, hga_prev)
            st_next = {}
            if c + 1 < nchunk:
                def mk_loadz(cc):
                    def f():
                        st_next["zc"] = load_z(cc)
                        st_next["e"] = pb.tile([128, NDC, TC, BSP], f16,
                                               tag="e", name="e")
                    return f

                def mk_make_e(dc):
                    def f():
                        nc.scalar.activation(
                            st_next["e"][:, dc], st_next["zc"][:, dc],
                            AF.Relu,
                            bias=bn_c[:, dc:dc + 1],
                            scale=bn_a[:, dc:dc + 1])
                    return f
                units.append(mk_loadz(c + 1))
                for dc in range(NDC):
                    units.append(mk_make_e(dc))
                # u-projection units for c+1 (need st_next["e"], so lazy)
                def mk_uproj_lazy(g, pc):
                    def f():
                        up = ps.tile([128, TC, BSP], f32, tag="u", bufs=2)
                        for dc in range(NDC):
                            nc.tensor.matmul(
                                up,
                                lhsT=wih[:, dc, g * 128:(g + 1) * 128],
                                rhs=st_next["e"][:, dc],
                                start=(dc == 0), stop=(dc == NDC - 1))
                        nc.scalar.activation(
                            ut2[pc][:, g], up, AF.Identity,
                            bias=bcc[:, g:g + 1])
                    return f
                for g in UPROJ_ORDER:
                    units.append(mk_uproj_lazy(g, (c + 1) % 2))

            pc = c % 2
            udone = 0
            hh_t = pb.tile([128, TC, NDC, BSP], f16, tag="hh", bufs=3,
                           name="hh")
            for tl in range(TC):
                si = c * TC + tl
                c_src = c_ab[si % 2]
                c_dst = c_ab[(si + 1) % 2]
                # bank-padded psum tiles (full 2KB each) per gate group
                gpg_f = ps.tile([128, 512], f32, tag="gg", bufs=1)
                gp_g = gpg_f[:, 0:NDC * BSP].rearrange(
                    "p (c b) -> p c b", c=NDC)
                gpif_f = ps.tile([128, 512], f32, tag="gif", bufs=1)
                gp_if = gpif_f[:, 0:2 * NDC * BSP].rearrange(
                    "p (c b) -> p c b", c=2 * NDC)
                gpo_f = ps.tile([128, 512], f32, tag="go", bufs=1)
                gp_o = gpo_f[:, 0:NDC * BSP].rearrange(
                    "p (c b) -> p c b", c=NDC)

                # u injection: one identity LDW, three copy-matmuls
                nc.tensor.matmul(gp_g, lhsT=ident,
                                 rhs=ut2[pc][:, 3 * NDC:4 * NDC, tl],
                                 start=True, stop=False)
                nc.tensor.matmul(gp_if, lhsT=ident,
                                 rhs=ut2[pc][:, 0:2 * NDC, tl],
                                 start=True, stop=False)
                nc.tensor.matmul(gp_o, lhsT=ident,
                                 rhs=ut2[pc][:, 2 * NDC:3 * NDC, tl],
                                 start=True, stop=False)

                def mmgrp(dst, glo, ghi):
                    for g in range(glo, ghi):
                        for dc in range(NDC):
                            nc.tensor.matmul(
                                dst[:, g - glo],
                                lhsT=whh[:, dc, g * 128:(g + 1) * 128],
                                rhs=h_prev[:, dc],
                                start=False,
                                stop=(g == ghi - 1 and dc == NDC - 1))
                mmgrp(gp_g, 3 * NDC, 4 * NDC)     # g-gate first
                tg = pb.tile([128, NDC, BSP], f32, tag="tg")
                nc.scalar.activation(tg, gp_g, AF.Tanh)
                mmgrp(gp_if, 0, 2 * NDC)          # i, f
                sgif = pb.tile([128, 2 * NDC, BSP], f32, tag="sgif")
                nc.scalar.activation(sgif, gp_if, AF.Sigmoid)
                t1 = pb.tile([128, NDC, BSP], f32, tag="t1")
                nc.vector.tensor_mul(t1, sgif[:, 0:NDC], tg)
                t2 = pb.tile([128, NDC, BSP], f32, tag="t2")
                nc.vector.tensor_mul(t2, sgif[:, NDC:2 * NDC], c_src)
                mmgrp(gp_o, 2 * NDC, 3 * NDC)     # o last
                nc.vector.tensor_add(c_dst, t1, t2)
                th = pb.tile([128, NDC, BSP], f32, tag="th")
                nc.scalar.activation(th, c_dst, AF.Tanh)
                sgo = pb.tile([128, NDC, BSP], f32, tag="sgo")
                nc.scalar.activation(sgo, gp_o, AF.Sigmoid)
                nc.vector.tensor_mul(hh_t[:, tl], sgo, th)
                h_prev = hh_t[:, tl]
                # interleave lagged/lookahead work into the dependency gap
                if tl >= 1:
                    target = (tl * len(units)) // (TC - 1)
                    while udone < target:
                        units[udone]()
                        udone += 1

            # -- pair h exchange for this chunk --
            db = c % 2
            nc.sync.dma_start(out=hin_d[db][:, :, :, :], in_=hh_t)
            if use_collective:
                nc.gpsimd.collective_compute(
                    "AllGather", ALU.bypass,
                    replica_groups=PAIRS,
                    ins=[hin_d[db][:, :, :, :]],
                    outs=[hga_d[db][:, :, :, :, :]])
            else:
                nc.sync.dma_start(out=hga_d[db][0], in_=hin_d[db][:, :, :, :])
                nc.sync.dma_start(out=hga_d[db][1], in_=hin_d[db][:, :, :, :])
            hga = pb.tile([128, 2, TC, NDC, BSP], f16, tag="hga")
            for s in range(2):
                nc.sync.dma_start(out=hga[:, s], in_=hga_d[db][s])

            # -- finish any remaining interleaved units --
            while udone < len(units):
                units[udone]()
                udone += 1
            hga_prev = hga

        for u in gating_units(nchunk - 1, hga_prev):
            u()

    nc.compile()
    return nc


def _prep_weights(i):
    """Host-side weight packing: fp16 casts, transposes, gate reorder.
    Returns (shared, per_stream[2]) dicts."""
    def perm_gates_rows(w):  # [4D, ...] rows (i,f,g,o) -> (i,f,o,g)
        return np.concatenate(
            [w[0:D], w[D:2 * D], w[3 * D:4 * D], w[2 * D:3 * D]], axis=0)

    f16 = np.float16
    shared = {}
    for s in range(2):
        shared[f"wg{s}T"] = np.ascontiguousarray(i[f"wg{s}"].T.astype(f16))
        shared[f"bg{s}c"] = i[f"bg{s}"].astype(np.float32)
    shared["wf1T"] = np.ascontiguousarray(i["wf1"].T.astype(f16))
    shared["wf2T"] = np.ascontiguousarray(i["wf2"].T.astype(f16))
    shared["bfc"] = i["bf"].astype(np.float32)

    per_stream = []
    for s in range(2):
        d = {}
        we = i[f"w_emb{s}"].T.astype(f16)           # [Fs, D]
        if we.shape[0] < FR:
            we = np.vstack([we, np.zeros((FR - we.shape[0], D), f16)])
        d["w_embST"] = np.ascontiguousarray(we)
        d["w_ihST"] = np.ascontiguousarray(
            perm_gates_rows(i[f"w_ih{s}"]).T.astype(f16))
        d["w_hhST"] = np.ascontiguousarray(
            perm_gates_rows(i[f"w_hh{s}"]).T.astype(f16))
        d["bcS"] = perm_gates_rows(
            (i[f"b_ih{s}"] + i[f"b_hh{s}"]).reshape(4 * D, 1))[:, 0].astype(
                np.float32)
        d["gammaS"] = i[f"gamma{s}"].astype(np.float32)
        d["betaS"] = i[f"beta{s}"].astype(np.float32)
        d["ssel"] = np.full(128, float(s), np.float32)
        d["nssel"] = np.full(128, 1.0 - float(s), np.float32)
        per_stream.append(d)
    return shared, per_stream


def _make_in_maps(inputs):
    shared, per_stream = _prep_weights(inputs)
    feats = (inputs["feat0"], inputs["feat1"])
    in_maps = []
    for cid in range(NCORES):
        parity = cid % 2
        pair = cid // 2
        m = dict(shared)
        m.update(per_stream[parity])
        psl = slice(pair * BSP, (pair + 1) * BSP)
        # [BSP, T, F] -> [F, T, BSP] fp16 (t-major columns), pad F to FR
        f = np.asarray(feats[parity][psl], np.float32)
        fT = np.ascontiguousarray(
            f.transpose(2, 1, 0).reshape(f.shape[2], -1).astype(np.float16))
        if fT.shape[0] < FR:
            fT = np.vstack(
                [fT, np.zeros((FR - fT.shape[0], fT.shape[1]), np.float16)])
        m["featST"] = fT
        in_maps.append(m)
    return in_maps


def _gather_out(res):
    """Reassemble [NCHUNK,128,NDC,TC,BS] f16 per core -> [B, T, D] f32."""
    outs = []
    for cid in range(NCORES):
        raw = res.results[cid]["out"]   # [NCHUNK, 128, NDC, TC, BS]
        # out[c, p, dc, t, b] -> [b, c*TC+t, dc*128+p]
        o = raw.transpose(4, 0, 3, 2, 1).reshape(BS, T, D)
        outs.append(np.asarray(o, np.float32))
    return np.concatenate(outs, axis=0)


def kernel(**inputs):
    from concourse.bass_utils import run_bass_kernel_spmd

    global _BUILT
    if _BUILT is None:
        _BUILT = _build(T)
    nc = _BUILT

    in_maps = _make_in_maps(inputs)
    res = run_bass_kernel_spmd(nc, in_maps, core_ids=list(range(NCORES)))
    return _gather_out(res)


if __name__ == "__main__":
    nc = _build(T)
    print("built ok")


# revision 17
# speedup vs baseline: 1.4839x; 1.0773x over previous
"""Trainium2 Bass kernel for the two-stream LSTM encoder.

Strategy (8 NeuronCores): cores are paired; each core runs ONE LSTM stream
for its pair's 32 batch rows (halves recurrent weight streaming). SPMD: the
program is identical on all cores; stream identity enters only through
per-core input tensors (ssel selects drive the few data-dependent spots).

  - Features are uploaded pre-transposed fp16 [F, T*BSP] (t-major columns)
    so Phase A is a straight matmul with no on-device transposes or casts.
  - Phase A: z = w_embS.T-style matmul into T-layout, spilled to DRAM fp16;
    BN stats all-reduced across the 8 cores into per-stream slots.
  - Phase B: time loop in chunks of 16 steps. Per step, the input
    projection u (precomputed per chunk) is injected into PSUM via an
    identity matmul so the recurrent matmuls accumulate on top of it —
    the post-matmul serial chain is just sigmoid(o) -> h mul. The mask is
    identically 1.0 (per the model's input spec) and is elided. Gate-group
    PSUM tiles are bank-padded to avoid read/write bank serialization.
  - Next chunk's u-projection and previous chunk's gating/fusion are
    interleaved into the per-step dependency gaps; gating elementwise runs
    on GpSimd so it never queues ahead of the critical-path DVE/ACT ops.
  - Output is stored in a raw chunk-major layout and reassembled on host.
Gate order is host-permuted from torch's (i,f,g,o) to (i,f,o,g).
"""

import os
import numpy as np

os.environ.setdefault("MYCRO_LOCAL_CACHE", "1")

NCORES = 8
B, T, D, F0, F1 = 128, 256, 512, 2048, 1024
FR = 2048                 # padded feature width (shared by both streams)
BS = B // NCORES          # 16 batch rows per core for output
BSP = 2 * BS              # 32 batch rows per pair (recurrence batch)
TC = 16                   # time steps per chunk
NCHUNK = T // TC
EPS = 1e-5
G4 = 4 * D                # 2048 gate dim
NDC = D // 128            # 4 d-chunks
NGC = G4 // 128           # 16 gate chunks
NFC = FR // 128           # 16 feature chunks

PAIRS = [[2 * i, 2 * i + 1] for i in range(NCORES // 2)]

_BUILT = None


def _build(t_steps=T, use_collective=True):
    import concourse.bass as bass
    import concourse.bacc as bacc
    import concourse.mybir as mybir
    import concourse.tile as tile
    from concourse.masks import make_identity
    from contextlib import ExitStack

    f16 = mybir.dt.float16
    f32 = mybir.dt.float32
    AF = mybir.ActivationFunctionType
    ALU = mybir.AluOpType

    nchunk = t_steps // TC
    rows2 = BSP * t_steps     # 8192 pair rows (t-major: r = t*BSP + b)
    nrc2 = rows2 // 512       # 16 row chunks of 512

    nc = bacc.Bacc(None, num_devices=NCORES)

    # ---------------- DRAM parameters ----------------
    featS = nc.declare_dram_parameter("featST", [FR, rows2], f16,
                                      isOutput=False)
    wembp = nc.declare_dram_parameter("w_embST", [FR, D], f16, isOutput=False)
    wihp = nc.declare_dram_parameter("w_ihST", [D, G4], f16, isOutput=False)
    whhp = nc.declare_dram_parameter("w_hhST", [D, G4], f16, isOutput=False)
    wg0T = nc.declare_dram_parameter("wg0T", [D, D], f16, isOutput=False)
    wg1T = nc.declare_dram_parameter("wg1T", [D, D], f16, isOutput=False)
    wf1T = nc.declare_dram_parameter("wf1T", [D, D], f16, isOutput=False)
    wf2T = nc.declare_dram_parameter("wf2T", [D, D], f16, isOutput=False)
    bcp = nc.declare_dram_parameter("bcS", [G4], f32, isOutput=False)
    bg0p = nc.declare_dram_parameter("bg0c", [D], f32, isOutput=False)
    bg1p = nc.declare_dram_parameter("bg1c", [D], f32, isOutput=False)
    bfp = nc.declare_dram_parameter("bfc", [D], f32, isOutput=False)
    gammap = nc.declare_dram_parameter("gammaS", [D], f32, isOutput=False)
    betap = nc.declare_dram_parameter("betaS", [D], f32, isOutput=False)
    sselp = nc.declare_dram_parameter("ssel", [128], f32, isOutput=False)
    nsselp = nc.declare_dram_parameter("nssel", [128], f32, isOutput=False)
    out_d = nc.declare_dram_parameter(
        "out", [nchunk, 128, NDC, TC, BS], f16, isOutput=True)

    # DRAM scratch
    zin = nc.dram_tensor("zin", [NDC, 128, rows2], f16)
    st_in = nc.dram_tensor("st_in", [128, 16], f32)
    st_out = nc.dram_tensor("st_out", [128, 16], f32)
    hin_d = [nc.dram_tensor(f"hin{i}", [128, TC, NDC, BSP], f16)
             for i in range(2)]
    hga_d = [nc.dram_tensor(f"hga{i}", [2, 128, TC, NDC, BSP], f16)
             for i in range(2)]

    inv_n = 1.0 / float(B * t_steps)
    if not use_collective:
        inv_n = 1.0 / float(BSP * t_steps)

    # warmup-collective scratch
    cw_in = nc.dram_tensor("cw_in", [128, 1], f32)
    cw_out = nc.dram_tensor("cw_out", [128, 1], f32)

    with tile.TileContext(nc) as tc, ExitStack() as stk:
        wp = stk.enter_context(tc.tile_pool(name="wp", bufs=1))

        # Warmup AllReduce: absorbs cross-core launch skew + CC first-call
        # cost inside Phase A's shadow so the real stats AR is fast.
        cw = wp.tile([128, 1], f32)
        nc.vector.memset(cw, 0.0)
        nc.gpsimd.dma_start(out=cw_in[:, :], in_=cw)
        if use_collective:
            nc.gpsimd.collective_compute(
                "AllReduce", mybir.AluOpType.add,
                replica_groups=[list(range(NCORES))],
                ins=[cw_in[:, :]], outs=[cw_out[:, :]])

        # ---------------- resident small tiles ----------------
        ident = wp.tile([128, 128], f16)
        make_identity(nc, ident)
        eps_t = wp.tile([128, 1], f32)
        nc.vector.memset(eps_t, 0.0)
        ssel = wp.tile([128, 1], f32)
        nc.sync.dma_start(out=ssel, in_=sselp.rearrange("(p o) -> p o", o=1))
        nssel = wp.tile([128, 1], f32)
        nc.sync.dma_start(out=nssel, in_=nsselp.rearrange("(p o) -> p o", o=1))

        def load_T(dram, nch, free):
            t = wp.tile([128, nch, free], f16, name=f"w_{dram.name}")
            nc.sync.dma_start(
                out=t, in_=dram.rearrange("(c p) g -> p c g", p=128))
            return t

        wih = load_T(wihp, NDC, G4)
        whh = load_T(whhp, NDC, G4)
        wg0 = load_T(wg0T, NDC, D)
        wg1 = load_T(wg1T, NDC, D)
        wf1 = load_T(wf1T, NDC, D)
        wf2 = load_T(wf2T, NDC, D)

        def load_col(dram, nch):
            t = wp.tile([128, nch], f32, name=f"b_{dram.name}")
            nc.sync.dma_start(
                out=t, in_=dram.rearrange("(g p) -> p g", p=128))
            return t

        bcc = load_col(bcp, NGC)     # cell bias, (i,f,o,g) column layout
        bgc = [load_col(bg0p, NDC), load_col(bg1p, NDC)]
        bfc = load_col(bfp, NDC)
        gamS = load_col(gammap, NDC)
        betS = load_col(betap, NDC)

        # per-rowgroup BN stat slots for MY stream: col = dc*nrc2 + rc
        stat_s = wp.tile([128, NDC * nrc2], f32, name="stat_s")
        stat_q = wp.tile([128, NDC * nrc2], f32, name="stat_q")

        # ---------------- Phase A: embed my stream (t-major cols) ---------
        with tc.tile_pool(name="pa", bufs=2) as pa, \
             tc.tile_pool(name="paw", bufs=1) as paw, \
             tc.tile_pool(name="psA", bufs=1, space="PSUM") as psA:
            wemb = paw.tile([128, NFC, D], f16)
            nc.sync.dma_start(
                out=wemb, in_=wembp.rearrange("(c p) g -> p c g", p=128))
            for rc in range(nrc2):
                fT = pa.tile([128, NFC, 512], f16, tag="fT")
                nc.gpsimd.dma_start(
                    out=fT,
                    in_=featS.rearrange("(c p) r -> p c r", p=128)[
                        :, :, rc * 512:(rc + 1) * 512])
                za = psA.tile([128, NDC, 512], f32, tag="za", bufs=2)
                for dc in range(NDC):
                    for fc in range(NFC):
                        nc.tensor.matmul(
                            za[:, dc],
                            lhsT=wemb[:, fc, dc * 128:(dc + 1) * 128],
                            rhs=fT[:, fc],
                            start=(fc == 0), stop=(fc == NFC - 1))
                zst = pa.tile([128, NDC, 512], f16, tag="zst")
                sq = pa.tile([128, 512], f32, tag="sq")
                for dc in range(NDC):
                    nc.scalar.activation(
                        zst[:, dc], za[:, dc],
                        AF.Identity,
                        accum_out=stat_s[:, dc * nrc2 + rc:
                                         dc * nrc2 + rc + 1])
                    nc.vector.tensor_tensor(
                        sq, za[:, dc], zst[:, dc], op=ALU.mult)
                    nc.vector.reduce_sum(
                        stat_q[:, dc * nrc2 + rc:dc * nrc2 + rc + 1],
                        sq, axis=mybir.AxisListType.X)
                nc.gpsimd.dma_start(
                    out=zin[:, :, rc * 512:(rc + 1) * 512].rearrange(
                        "c p r -> p c r"),
                    in_=zst)

        # ---------------- BN stats allreduce (per-stream slots) -----------
        ared = wp.tile([128, 8], f32)
        nc.vector.reduce_sum(
            ared[:, 0:4],
            stat_s.rearrange("p (c r) -> p c r", c=NDC),
            axis=mybir.AxisListType.X)
        nc.vector.reduce_sum(
            ared[:, 4:8],
            stat_q.rearrange("p (c r) -> p c r", c=NDC),
            axis=mybir.AxisListType.X)
        # slot 0:8 <- stream0 contribution, 8:16 <- stream1 (ssel-masked)
        sts = wp.tile([128, 16], f32)
        nc.scalar.activation(sts[:, 0:8], ared, AF.Identity,
                             scale=nssel[:, 0:1])
        nc.scalar.activation(sts[:, 8:16], ared, AF.Identity,
                             scale=ssel[:, 0:1])
        nc.gpsimd.dma_start(out=st_in[:, :], in_=sts)
        if use_collective:
            nc.gpsimd.collective_compute(
                "AllReduce", ALU.add,
                replica_groups=[list(range(NCORES))],
                ins=[st_in[:, :]], outs=[st_out[:, :]])
        else:
            nc.gpsimd.dma_start(out=st_out[:, :], in_=st_in[:, :])
        ag = wp.tile([128, 16], f32)
        nc.gpsimd.dma_start(out=ag, in_=st_out[:, :])

        # my stream's stats: agm = ag[s0] + ssel * (ag[s1] - ag[s0])
        d8 = wp.tile([128, 8], f32)
        nc.vector.tensor_sub(d8, ag[:, 8:16], ag[:, 0:8])
        agm = wp.tile([128, 8], f32)
        nc.vector.scalar_tensor_tensor(
            agm, d8, ssel[:, 0:1], ag[:, 0:8], op0=ALU.mult, op1=ALU.add)

        # a = gamma / sqrt(var+eps), c = beta - mu * a
        mu = wp.tile([128, NDC], f32)
        nc.vector.tensor_scalar_mul(mu, agm[:, 0:4], inv_n)
        var = wp.tile([128, NDC], f32)
        nc.vector.tensor_scalar_mul(var, agm[:, 4:8], inv_n)
        musq = wp.tile([128, NDC], f32)
        nc.vector.tensor_mul(musq, mu, mu)
        nc.vector.tensor_sub(var, var, musq)
        nc.vector.tensor_scalar_add(var, var, EPS)
        sig = wp.tile([128, NDC], f32)
        nc.scalar.activation(sig, var, AF.Sqrt, bias=eps_t[:, 0:1])
        isig = wp.tile([128, NDC], f32)
        nc.vector.reciprocal(isig, sig)
        bn_a = wp.tile([128, NDC], f32)
        nc.vector.tensor_mul(bn_a, gamS, isig)
        bn_c = wp.tile([128, NDC], f32)
        nc.vector.tensor_mul(bn_c, mu, bn_a)
        nc.vector.tensor_sub(bn_c, betS, bn_c)

        # ---------------- Phase B: recurrence (one stream, 32 batch) ------
        pb = stk.enter_context(tc.tile_pool(name="pb", bufs=2))
        ps = stk.enter_context(tc.tile_pool(name="ps", bufs=2, space="PSUM"))

        h_zero = wp.tile([128, NDC, BSP], f16)
        nc.vector.memset(h_zero, 0.0)
        # c-state ping-pong (mask==1 so no separate masked copy is needed)
        c_ab = [wp.tile([128, NDC, BSP], f32, name=f"c{i}") for i in range(2)]
        nc.vector.memset(c_ab[0], 0.0)
        nc.vector.memset(c_ab[1], 0.0)
        # u double-buffer by chunk parity: [128, NGC, TC, BSP] fp16
        ut2 = [wp.tile([128, NGC, TC, BSP], f16, name=f"ut{i}")
               for i in range(2)]

        UPROJ_ORDER = list(range(3 * NDC, 4 * NDC)) + list(range(0, 3 * NDC))

        def load_z(c):
            zc = pb.tile([128, NDC, TC, BSP], f16, tag="zc")
            nc.gpsimd.dma_start(
                out=zc,
                in_=zin.rearrange("c p r -> p c r")[
                    :, :, c * TC * BSP:(c + 1) * TC * BSP])
            return zc

        def make_e(zc):
            e = pb.tile([128, NDC, TC, BSP], f16, tag="e")
            for dc in range(NDC):
                nc.scalar.activation(
                    e[:, dc], zc[:, dc], AF.Relu,
                    bias=bn_c[:, dc:dc + 1],
                    scale=bn_a[:, dc:dc + 1])
            return e

        def uproj_units(e_ref, pc):
            """u = e @ w_ih.T + bc for one chunk into ut2[pc].
            e_ref: dict with key "e" (filled lazily). Emits (kind, fn)."""
            units = []
            stu = {}

            def mk_mm(g):
                def f():
                    up = ps.tile([128, TC, BSP], f32, tag="u", bufs=2,
                                 name="up")
                    for dc in range(NDC):
                        nc.tensor.matmul(
                            up,
                            lhsT=wih[:, dc, g * 128:(g + 1) * 128],
                            rhs=e_ref["e"][:, dc],
                            start=(dc == 0), stop=(dc == NDC - 1))
                    stu[g] = up
                return f

            def mk_act(g):
                def f():
                    nc.scalar.activation(
                        ut2[pc][:, g], stu[g], AF.Identity,
                        bias=bcc[:, g:g + 1])
                return f
            for g in UPROJ_ORDER:
                units.append(("pe", mk_mm(g)))
                units.append(("ot", mk_act(g)))
            return units

        def gating_units(c, hga):
            """Gating/fusion/output for chunk c (uses chunk c's h, gathered
            from the pair). All elementwise is split per-dc (~256 cols per
            op) so no unit can queue-delay the per-step critical-path
            DVE/ACT ops by more than ~0.2us."""
            units = []
            st = {}

            def mk_sel_alloc(s):
                def f():
                    st[f"hd{s}"] = pb.tile([128, TC, NDC, BS], f16,
                                           tag=f"hd{s}", bufs=1,
                                           name=f"hd{s}")
                    st[f"hs{s}"] = pb.tile([128, TC, NDC, BS], f16,
                                           tag=f"hs{s}", name=f"hs{s}")
                return f

            def mk_sel(s, dc):
                def f():
                    hd = st[f"hd{s}"]
                    hs = st[f"hs{s}"]
                    nc.vector.tensor_tensor(
                        hd[:, :, dc], hga[:, s, :, dc, BS:BSP],
                        hga[:, s, :, dc, 0:BS], op=ALU.subtract)
                    nc.vector.scalar_tensor_tensor(
                        hs[:, :, dc], hd[:, :, dc], ssel[:, 0:1],
                        hga[:, s, :, dc, 0:BS],
                        op0=ALU.mult, op1=ALU.add)
                return f
            for s in range(2):
                units.append(("ot", mk_sel_alloc(s)))
                for dc in range(NDC):
                    units.append(("ot", mk_sel(s, dc)))

            def mk_pg_alloc(s):
                def f():
                    st[f"pg{s}"] = ps.tile([128, NDC, TC, BS], f32,
                                           tag="lag", name=f"pg{s}",
                                           bufs=1)
                return f

            def mk_gate_mm(s, go):
                def f():
                    src = st[f"hs{1 - s}"]
                    wgT = (wg0, wg1)[s]
                    pg = st[f"pg{s}"]
                    for dc in range(NDC):
                        nc.tensor.matmul(
                            pg[:, go],
                            lhsT=wgT[:, dc, go * 128:(go + 1) * 128],
                            rhs=src[:, :, dc, :],
                            start=(dc == 0), stop=(dc == NDC - 1))
                return f

            def mk_fin_alloc(s):
                def f():
                    st[f"sp{s}"] = pb.tile([128, NDC, TC, BS], f16,
                                           tag=f"sp{s}", name=f"sp{s}")
                    st[f"o{s}"] = pb.tile([128, NDC, TC, BS], f16,
                                          tag=f"o{s}", name=f"o{s}")
                return f

            def mk_gate_fin(s, dc):
                def f():
                    sp = st[f"sp{s}"]
                    nc.scalar.activation(
                        sp[:, dc], st[f"pg{s}"][:, dc], AF.Sigmoid,
                        bias=bgc[s][:, dc:dc + 1])
                    nc.vector.tensor_tensor(
                        st[f"o{s}"][:, dc], sp[:, dc],
                        st[f"hs{s}"][:, :, dc], op=ALU.mult)
                return f

            for s in range(2):
                units.append(("ot", mk_pg_alloc(s)))
                for go in range(NDC):
                    units.append(("pe", mk_gate_mm(s, go)))
                units.append(("ot", mk_fin_alloc(s)))
                for dc in range(NDC):
                    units.append(("ot", mk_gate_fin(s, dc)))

            def fp_alloc():
                st["fp"] = ps.tile([128, NDC, TC, BS], f32, tag="lag",
                                   name="fp", bufs=1)
            units.append(("ot", fp_alloc))

            def mk_fusion(do, s):
                def f():
                    wfT = (wf1, wf2)[s]
                    for dc in range(NDC):
                        nc.tensor.matmul(
                            st["fp"][:, do],
                            lhsT=wfT[:, dc, do * 128:(do + 1) * 128],
                            rhs=st[f"o{s}"][:, dc],
                            start=(s == 0 and dc == 0),
                            stop=(s == 1 and dc == NDC - 1))
                return f
            for do in range(NDC):
                for s in range(2):
                    units.append(("pe", mk_fusion(do, s)))

            def otn_alloc():
                st["otn"] = pb.tile([128, NDC, TC, BS], f16, tag="otn",
                                    name="otn")
            units.append(("ot", otn_alloc))

            def mk_otn(dc):
                def f():
                    nc.scalar.activation(
                        st["otn"][:, dc], st["fp"][:, dc], AF.Tanh,
                        bias=bfc[:, dc:dc + 1])
                return f
            for dc in range(NDC):
                units.append(("ot", mk_otn(dc)))

            def store():
                nc.gpsimd.dma_start(out=out_d[c], in_=st["otn"])
            units.append(("ot", store))
            return units

        # ---- chunk 0 prologue: z, e, u (dense) ----
        zc_cur = load_z(0)
        e_cur = make_e(zc_cur)
        for _, u in uproj_units({"e": e_cur}, 0):
            u()

        hga_prev = None
        h_prev = h_zero
        for c in range(nchunk):
            units = []
            if hga_prev is not None:
                units += gating_units(c - 1, hga_prev)
            st_next = {}
            if c + 1 < nchunk:
                def mk_loadz(cc):
                    def f():
                        st_next["zc"] = load_z(cc)
                        st_next["e"] = pb.tile([128, NDC, TC, BSP], f16,
                                               tag="e", name="e")
                    return f

                def mk_make_e(dc):
                    def f():
                        nc.scalar.activation(
                            st_next["e"][:, dc], st_next["zc"][:, dc],
                            AF.Relu,
                            bias=bn_c[:, dc:dc + 1],
                            scale=bn_a[:, dc:dc + 1])
                    return f
                units.append(("ot", mk_loadz(c + 1)))
                for dc in range(NDC):
                    units.append(("ot", mk_make_e(dc)))
                units += uproj_units(st_next, (c + 1) % 2)

            pc = c % 2
            # single ordered unit queue; the mid-step slot (between the
            # if-group and o-group matmuls, where the c-chain latency hides
            # PE work) only takes the next unit when it is PE-heavy
            udone = 0
            hh_t = pb.tile([128, TC, NDC, BSP], f16, tag="hh", bufs=3,
                           name="hh")
            for tl in range(TC):
                si = c * TC + tl
                c_src = c_ab[si % 2]
                c_dst = c_ab[(si + 1) % 2]
                # bank-padded psum tiles (full 2KB each) per gate group;
                # gp_o is double-buffered: the next step's o-matmuls must
                # not wait for this step's sigmoid read
                gpg_f = ps.tile([128, 512], f32, tag="gg", bufs=1)
                gp_g = gpg_f[:, 0:NDC * BSP].rearrange(
                    "p (c b) -> p c b", c=NDC)
                gpif_f = ps.tile([128, 512], f32, tag="gif", bufs=1)
                gp_if = gpif_f[:, 0:2 * NDC * BSP].rearrange(
                    "p (c b) -> p c b", c=2 * NDC)
                gpo_f = ps.tile([128, 512], f32, tag="go", bufs=2)
                gp_o = gpo_f[:, 0:NDC * BSP].rearrange(
                    "p (c b) -> p c b", c=NDC)

                # u injection: identity copy-matmuls (exact in fp16)
                nc.tensor.matmul(gp_g, lhsT=ident,
                                 rhs=ut2[pc][:, 3 * NDC:4 * NDC, tl],
                                 start=True, stop=False)
                nc.tensor.matmul(gp_if, lhsT=ident,
                                 rhs=ut2[pc][:, 0:2 * NDC, tl],
                                 start=True, stop=False)
                nc.tensor.matmul(gp_o, lhsT=ident,
                                 rhs=ut2[pc][:, 2 * NDC:3 * NDC, tl],
                                 start=True, stop=False)

                def mmgrp(dst, glo, ghi):
                    # dc-outer so the first matmuls need only h[:, 0],
                    # which the split h-mul below produces first
                    for dc in range(NDC):
                        for g in range(glo, ghi):
                            nc.tensor.matmul(
                                dst[:, g - glo],
                                lhsT=whh[:, dc, g * 128:(g + 1) * 128],
                                rhs=h_prev[:, dc],
                                start=False,
                                stop=(g == ghi - 1 and dc == NDC - 1))
                mmgrp(gp_g, 3 * NDC, 4 * NDC)     # g-gate first
                tg = pb.tile([128, NDC, BSP], f32, tag="tg")
                nc.scalar.activation(tg, gp_g, AF.Tanh)
                mmgrp(gp_if, 0, 2 * NDC)          # i, f
                sgif = pb.tile([128, 2 * NDC, BSP], f32, tag="sgif")
                nc.scalar.activation(sgif, gp_if, AF.Sigmoid)
                t1 = pb.tile([128, NDC, BSP], f32, tag="t1")
                nc.vector.tensor_mul(t1, sgif[:, 0:NDC], tg)
                t2 = pb.tile([128, NDC, BSP], f32, tag="t2")
                nc.vector.tensor_mul(t2, sgif[:, NDC:2 * NDC], c_src)
                # mid-step PE fill: hides the c-chain so th is ready
                # before the o-group finishes
                if tl >= 1 and udone < len(units) and \
                        units[udone][0] == "pe":
                    units[udone][1]()
                    udone += 1
                mmgrp(gp_o, 2 * NDC, 3 * NDC)     # o last
                nc.vector.tensor_add(c_dst, t1, t2)
                th = pb.tile([128, NDC, BSP], f32, tag="th")
                nc.scalar.activation(th, c_dst, AF.Tanh)
                sgo = pb.tile([128, NDC, BSP], f32, tag="sgo")
                # split sigmoid(o) + h-mul in halves: h[:, 0:2] lands
                # early so the next step's dc-outer matmuls start sooner
                nc.scalar.activation(sgo[:, 0:2], gp_o[:, 0:2], AF.Sigmoid)
                nc.vector.tensor_mul(hh_t[:, tl, 0:2], sgo[:, 0:2],
                                     th[:, 0:2])
                nc.scalar.activation(sgo[:, 2:4], gp_o[:, 2:4], AF.Sigmoid)
                nc.vector.tensor_mul(hh_t[:, tl, 2:4], sgo[:, 2:4],
                                     th[:, 2:4])
                h_prev = hh_t[:, tl]
                # remaining units after the critical-path ops
                if tl >= 1:
                    target = (tl * len(units)) // (TC - 1)
                    while udone < target:
                        if units[udone][0] == "pe" and udone >= target - 1:
                            break   # save for the next mid-step slot
                        units[udone][1]()
                        udone += 1

            # -- pair h exchange for this chunk --
            db = c % 2
            nc.sync.dma_start(out=hin_d[db][:, :, :, :], in_=hh_t)
            if use_collective:
                nc.gpsimd.collective_compute(
                    "AllGather", ALU.bypass,
                    replica_groups=PAIRS,
                    ins=[hin_d[db][:, :, :, :]],
                    outs=[hga_d[db][:, :, :, :, :]])
            else:
                nc.sync.dma_start(out=hga_d[db][0], in_=hin_d[db][:, :, :, :])
                nc.sync.dma_start(out=hga_d[db][1], in_=hin_d[db][:, :, :, :])
            hga = pb.tile([128, 2, TC, NDC, BSP], f16, tag="hga")
            for s in range(2):
                nc.sync.dma_start(out=hga[:, s], in_=hga_d[db][s])

            # -- finish any remaining interleaved units --
            while udone < len(units):
                units[udone][1]()
                udone += 1
            hga_prev = hga

        for _, u in gating_units(nchunk - 1, hga_prev):
            u()

    nc.compile()
    return nc


def _prep_weights(i):
    """Host-side weight packing: fp16 casts, transposes, gate reorder.
    Returns (shared, per_stream[2]) dicts."""
    def perm_gates_rows(w):  # [4D, ...] rows (i,f,g,o) -> (i,f,o,g)
        return np.concatenate(
            [w[0:D], w[D:2 * D], w[3 * D:4 * D], w[2 * D:3 * D]], axis=0)

    f16 = np.float16
    shared = {}
    for s in range(2):
        shared[f"wg{s}T"] = np.ascontiguousarray(i[f"wg{s}"].T.astype(f16))
        shared[f"bg{s}c"] = i[f"bg{s}"].astype(np.float32)
    shared["wf1T"] = np.ascontiguousarray(i["wf1"].T.astype(f16))
    shared["wf2T"] = np.ascontiguousarray(i["wf2"].T.astype(f16))
    shared["bfc"] = i["bf"].astype(np.float32)

    per_stream = []
    for s in range(2):
        d = {}
        we = i[f"w_emb{s}"].T.astype(f16)           # [Fs, D]
        if we.shape[0] < FR:
            we = np.vstack([we, np.zeros((FR - we.shape[0], D), f16)])
        d["w_embST"] = np.ascontiguousarray(we)
        d["w_ihST"] = np.ascontiguousarray(
            perm_gates_rows(i[f"w_ih{s}"]).T.astype(f16))
        d["w_hhST"] = np.ascontiguousarray(
            perm_gates_rows(i[f"w_hh{s}"]).T.astype(f16))
        d["bcS"] = perm_gates_rows(
            (i[f"b_ih{s}"] + i[f"b_hh{s}"]).reshape(4 * D, 1))[:, 0].astype(
                np.float32)
        d["gammaS"] = i[f"gamma{s}"].astype(np.float32)
        d["betaS"] = i[f"beta{s}"].astype(np.float32)
        d["ssel"] = np.full(128, float(s), np.float32)
        d["nssel"] = np.full(128, 1.0 - float(s), np.float32)
        per_stream.append(d)
    return shared, per_stream


def _make_in_maps(inputs):
    shared, per_stream = _prep_weights(inputs)
    feats = (inputs["feat0"], inputs["feat1"])
    in_maps = []
    for cid in range(NCORES):
        parity = cid % 2
        pair = cid // 2
        m = dict(shared)
        m.update(per_stream[parity])
        psl = slice(pair * BSP, (pair + 1) * BSP)
        # [BSP, T, F] -> [F, T, BSP] fp16 (t-major columns), pad F to FR
        f = np.asarray(feats[parity][psl], np.float32)
        fT = np.ascontiguousarray(
            f.transpose(2, 1, 0).reshape(f.shape[2], -1).astype(np.float16))
        if fT.shape[0] < FR:
            fT = np.vstack(
                [fT, np.zeros((FR - fT.shape[0], fT.shape[1]), np.float16)])
        m["featST"] = fT
        in_maps.append(m)
    return in_maps


def _gather_out(res):
    """Reassemble [NCHUNK,128,NDC,TC,BS] f16 per core -> [B, T, D] f32."""
    outs = []
    for cid in range(NCORES):
        raw = res.results[cid]["out"]   # [NCHUNK, 128, NDC, TC, BS]
        # out[c, p, dc, t, b] -> [b, c*TC+t, dc*128+p]
        o = raw.transpose(4, 0, 3, 2, 1).reshape(BS, T, D)
        outs.append(np.asarray(o, np.float32))
    return np.concatenate(outs, axis=0)


def kernel(**inputs):
    from concourse.bass_utils import run_bass_kernel_spmd

    global _BUILT
    if _BUILT is None:
        _BUILT = _build(T)
    nc = _BUILT

    in_maps = _make_in_maps(inputs)
    res = run_bass_kernel_spmd(nc, in_maps, core_ids=list(range(NCORES)))
    return _gather_out(res)


if __name__ == "__main__":
    nc = _build(T)
    print("built ok")
